# revision 132
# baseline (speedup 1.0000x reference)
"""MemNet (scatter_memory) Trainium2 kernel, v2.

Model (per batch row b):
  memory   = emb[context_x[b]]                    # [L, D] gather
  v_aspect = masked-mean(emb[target_x[b]])        # [D]
  v_loc    = 1 - |pos - target_loc[b]| / context_len[b]
  3 hops of: scores = tanh((memory*v_loc) @ w_mem + vec@w_vec + b)
             alpha  = masked softmax;  vec = alpha @ (memory*v_loc) + vec@lin_w+lin_b
  logits   = vec @ out_w + out_b

Sharding: data-parallel over batch, 32 rows per core on 8 cores; the fp16
embedding table is index-compacted per core and fetched by indirect DMA
gather.

Key structure (vs v1):
- The content score emb@w_mem is a pure weight transform; it is appended
  as column 300 of the gathered rows (inside the 768B alignment padding),
  so scores arrive with the gather for free.
- Attention contraction runs transposed on the PE: per (chunk, d-slice),
  stationary = mem[128, DK], moving = the chunk's alpha column [128, 1],
  accumulating over each batch row's 4 chunks directly into vec^T layout
  [DK, b] in PSUM. Cost ~ 1 cycle per matmul (output free size 1).
- v_loc/cmask are host-side index-derived tensors; hop 1 (whose alpha
  depends only on v_aspect) is pipelined per gather group so its attention
  runs under the gather phase; the softmax denominator accumulates per
  group in an open PSUM matmul group.
"""

import numpy as np

import concourse.bass as bass
import concourse.bacc as bacc
import concourse.mybir as mybir
import concourse.tile as tile
from concourse import bass_utils

N_CORES = 8
B, L, T, V, D, C = 256, 512, 5, 50000, 300, 3
N_HOPS = 3
BP = B // N_CORES          # 32 batch rows per core
P = 128                    # partitions
NCH = (BP * L) // P        # 128 chunk columns; chunk c: b=c//4, l=(c%4)*128+p
CPB = L // P               # 4 chunks per batch row
NGRP = 16                  # gather groups (<=1024 idxs per dma_gather)
GW = NCH // NGRP           # chunk columns per gather group (8)
DK = [128, 128, 44]        # D split across PSUM partition chunks
DKP = [128, 128, 128]      # k=2 padded to 128 (pad rows are dead/zero) so
                           # PSUM tiles are fully written -> one big readout
DOF = [0, 128, 256]
TCOL = (BP * T + P - 1) // P  # 2 columns of host-provided target rows
EPAD16 = 384               # row length in f16 units (768B rows: 300 fp16
                           # values + packed per-row weight transforms)
SCORE_COL = 300            # f16 column of the packed content score emb@w_mem
OW_COL = 304               # [304:307) packed output scores emb@out_w
WV_COL = 307               # packed emb@w_vec (feeds the next hop's svec)
TE = 304                   # target row pad (f16 units)
U_PAD = 16768              # fixed local-table rows (>= 16384)

F16 = mybir.dt.float16
I16 = mybir.dt.int16
F32 = mybir.dt.float32
F8 = mybir.dt.float8e4

# auxf (f32) column layout
AF_TLEN = 0            # target_len per partition (p % 32)
AF_T0 = 1              # target t-index for j=0 (p // 32)
AF_T1 = 2              # target t-index for j=1 (4 + p // 32)
AF_ID32 = 3            # [3:35) id32 (rows 0:32)
AF_OUTB = 35           # effective out bias (rows 0:3)
AF_ATTNB = 36          # attn_b (row 0)
AF_TLENP = 37          # target_len per slot (rows 0:32)
AF_SSEL = 38           # [38:70) ssel: target row p -> slot column
AF_LWVB = 70           # lin_b @ w_vec + attn_b (row 0)
AF_N = 71

# aux16 (f16) column layout
A6_GSEL = 0            # [0:32)   gsel: chunk c (partition) -> b
A6_SSEL = 32           # [32:64)  ssel: target row p -> b = p % 32
A6_WVEC = 64           # [64:67)  w_vec d-chunks
A6_ONES = 67           # ones column
A6_ONESR = 68          # [68:196) ones row (partition 0)
A6_OUTW = 196          # [196:205) out_w  [d-part, 3 k, C]
A6_LINB = 205          # [205:589) lin_b as an f16 row (partition 0)
A6_LWOW = 589          # [589:598) lin_w @ out_w  [d-part, 3 k, C]
A6_VLOC = 598          # [598:726) vloc
A6_CMASK = 726         # [726:854) cmask
A6_CV = 854            # [854:982) cmask * vloc
A6_OUTB = 982          # [982:985) effective out bias row (partition 0)
A6_LWV = 985           # [985:988) lin_w @ w_vec d-chunks
A6_N = 988


def _ap2d(tile_ap, col_off, stride, n):
    """2D AP over a 3D tile: partition dim + one strided free dim."""
    return bass.AP(tile_ap.tensor, tile_ap.offset + col_off,
                   [list(tile_ap.ap[0]), [stride, n]])


def _row_rep4(ap2):
    """[1, BP] row AP -> [1, CPB, BP] with the outer dim broadcast (step 0),
    so column c = 32*r + j reads value j."""
    return bass.AP(ap2.tensor, ap2.offset,
                   [list(ap2.ap[0]), [0, CPB], [1, BP]])


DEBUG = False


def build_module(m=(BP, BP, BP)):
    """m = (m1, m2, m3): valid slot count per l-band r=1..3 (band 0 is always
    full). Chunk column c = 32*r + j holds l in [128r, 128r+128) of the
    batch in slot j (host sorts batches by descending context_len, so valid
    chunks are a prefix of each band); only valid chunks are gathered.
    """
    m_band = [BP, m[0], m[1], m[2]]
    # per-gather-group valid widths (group g = columns [8g, 8g+8))
    gw = [max(0, min(GW, m_band[(8 * g) // BP] - (8 * g) % BP))
          for g in range(NGRP)]
    vcols = [8 * g + cc for g in range(NGRP) for cc in range(gw[g])]

    nc = bacc.Bacc("TRN2", target_bir_lowering=False, debug=False,
                   num_devices=N_CORES)

    emb_d = nc.dram_tensor("emb_loc", [U_PAD, EPAD16], F16,
                           kind="ExternalInput")
    NIC = GW * P // 16  # idx tile columns per full group
    NSPL = 4 * NIC      # groups 0..3 arrive in a small first DMA
    ctx_idxa_d = nc.dram_tensor("ctx_idx16a", [P, NSPL], I16,
                                kind="ExternalInput")
    ctx_idxb_d = nc.dram_tensor("ctx_idx16b", [P, NCH * P // 16 - NSPL], I16,
                                kind="ExternalInput")
    tgtr_d = nc.dram_tensor("tgtr_h", [P, TCOL * TE], F16,
                            kind="ExternalInput")
    auxf_d = nc.dram_tensor("auxf_h", [P, AF_N], F32, kind="ExternalInput")
    aux16_d = nc.dram_tensor("aux16_h", [P, A6_N], F16, kind="ExternalInput")
    linw_d = nc.dram_tensor("lin_w_h", [P, 3 * 384], F16, kind="ExternalInput")

    out_d = nc.dram_tensor("logits_t", [C, BP], F32, kind="ExternalOutput")
    if DEBUG:
        dbg_msv_d = nc.dram_tensor("dbg_msv", [P, NCH], F32,
                                   kind="ExternalOutput")
        dbg_em_d = nc.dram_tensor("dbg_em", [P, NCH], F32,
                                  kind="ExternalOutput")
        dbg_va_d = nc.dram_tensor("dbg_va", [BP, D], F32,
                                  kind="ExternalOutput")
        dbg_v1_d = nc.dram_tensor("dbg_v1", [P, 3 * BP], F32,
                                  kind="ExternalOutput")
        dbg_v2_d = nc.dram_tensor("dbg_v2", [P, 3 * BP], F32,
                                  kind="ExternalOutput")

    mult = mybir.AluOpType.mult
    addop = mybir.AluOpType.add
    is_lt = mybir.AluOpType.is_lt
    AF = mybir.ActivationFunctionType

    with tile.TileContext(nc) as tc:
        with (
            tc.tile_pool(name="sb", bufs=1) as sb,
            tc.tile_pool(name="sc", bufs=4) as scr,
            tc.tile_pool(name="ps", bufs=1, space="PSUM") as ps,
            tc.tile_pool(name="ps3", bufs=2, space="PSUM") as ps3,
        ):
            # ---- persistent SBUF tiles ----
            idxa_sb = sb.tile([P, NSPL], I16, tag="idxa")
            idxb_sb = sb.tile([P, NCH * P // 16 - NSPL], I16, tag="idxb")
            auxf_sb = sb.tile([P, AF_N], F32, tag="auxf")
            aux16_sb = sb.tile([P, A6_N], F16, tag="aux16")
            linw_sb = sb.tile([P, 3, 384], F16, tag="linw")
            mem_sb = [sb.tile([P, GW, EPAD16], F16, tag=f"mem{g}",
                              name=f"mem{g}") for g in range(NGRP)]
            tgtr_sb = sb.tile([P, TCOL, TE], F16, tag="tgtr")

            tmask = sb.tile([P, TCOL], F32, tag="tmask")
            a0 = sb.tile([P, BP, TCOL], F16, tag="a0")
            tlenr = sb.tile([BP, 1], F32, tag="tlenr")
            va_sb = sb.tile([BP, D], F32, tag="va")
            vecT_a = sb.tile([P, 3, BP], F16, tag="vecTa", name="vecT_a")
            vecT_b = sb.tile([P, 3, BP], F16, tag="vecTb", name="vecT_b")
            msv = sb.tile([P, NCH], F32, tag="msv")
            sc_f = sb.tile([P, NCH], F32, tag="scf")
            e_m = sb.tile([P, NCH], F16, tag="em")
            aw = sb.tile([P, NCH], F16, tag="aw")
            awn = sb.tile([P, NCH], F16, tag="awn")
            aw1 = [sb.tile([P, GW], F16, tag=f"aw1_{g}", name=f"aw1_{g}")
                   for g in range(NGRP)]
            svec_sb = sb.tile([1, BP], F16, tag="svec")
            csc_sb = sb.tile([P, 1], F16, tag="cs")
            rdr_sb = sb.tile([1, BP], F16, tag="rdr")
            rd_sb = sb.tile([P, BP], F32, tag="rdbc_s")
            asm3 = sb.tile([P, 3, BP], F32, tag="asm")
            lg_sb = sb.tile([C, BP], F32, tag="lg")

            vloc_ap = aux16_sb[:, A6_VLOC:A6_VLOC + NCH]
            cmask_ap = aux16_sb[:, A6_CMASK:A6_CMASK + NCH]
            cv_ap = aux16_sb[:, A6_CV:A6_CV + NCH]
            gsel_ap = aux16_sb[:, A6_GSEL:A6_GSEL + BP]
            ssel_ap = auxf_sb[:, AF_SSEL:AF_SSEL + BP]
            ones_ap = aux16_sb[:, A6_ONES:A6_ONES + 1]
            onesr_ap = aux16_sb[0:1, A6_ONESR:A6_ONESR + P]
            id32_ap = auxf_sb[0:BP, AF_ID32:AF_ID32 + BP]

            # ---- input DMAs (context indices first so gathers start early) ----
            nc.sync.dma_start(idxa_sb[:], ctx_idxa_d.ap())
            nc.sync.dma_start(idxb_sb[:], ctx_idxb_d.ap())
            nc.sync.dma_start(tgtr_sb[:], tgtr_d.ap())
            nc.sync.dma_start(auxf_sb[:], auxf_d.ap())
            nc.sync.dma_start(aux16_sb[:], aux16_d.ap())

            # uninvolved score slots must stay finite: zero msv/e_m once so
            # chunks never gathered (invalid/pad) read as 0 through the
            # softmax (cmask/cv are 0 there host-side)
            nc.vector.memset(msv[:], 0.0)
            nc.vector.memset(e_m[:], 0.0)

            # ---- gathers (768B rows: fp16 values + fp16 score), only the
            # valid prefix of each group ----
            for g in range(NGRP):
                if gw[g] == 0:
                    continue
                nig = gw[g] * P
                if g < 4:
                    iap = idxa_sb[:, g * NIC:g * NIC + nig // 16]
                else:
                    g4 = g - 4
                    iap = idxb_sb[:, g4 * NIC:g4 * NIC + nig // 16]
                nc.gpsimd.dma_gather(
                    out_ap=mem_sb[g][:, 0:gw[g], :], in_ap=emb_d.ap(),
                    idxs_ap=iap, num_idxs=nig, num_idxs_reg=nig,
                    elem_size=EPAD16)

            # lin_w (294KB) is only needed for hop-1's assembly (~31us), but
            # its DMA would steal gather-rail time. Gate it behind the last
            # gather with a 1-element copy (WAW on linw_sb) so all gathers
            # finish ~1.3us earlier.
            glast = max(g for g in range(NGRP) if gw[g] > 0)
            nc.gpsimd.tensor_copy(out=linw_sb[0:1, 0, 0:1],
                                  in_=mem_sb[glast][0:1, 0, 0:1])
            nc.sync.dma_start(linw_sb[:], linw_d.ap())

            # ---- v_aspect -> vecT_a ----
            nc.vector.tensor_tensor(
                out=tmask[:], in0=auxf_sb[:, AF_T0:AF_T0 + TCOL],
                in1=auxf_sb[:, AF_TLEN:AF_TLEN + 1].to_broadcast([P, TCOL]),
                op=is_lt)
            va_ps = ps.tile([BP, D], F32, tag="acc300", space="PSUM")
            for j in range(TCOL):
                nc.vector.tensor_scalar_mul(a0[:, :, j], ssel_ap,
                                            tmask[:, j:j + 1])
                nc.tensor.matmul(va_ps[:], lhsT=a0[:, :, j],
                                 rhs=tgtr_sb[:, j, 0:D],
                                 start=(j == 0), stop=(j == TCOL - 1))
            nc.vector.reciprocal(tlenr[:], auxf_sb[0:BP, AF_TLENP:AF_TLENP + 1])
            nc.vector.tensor_scalar_mul(va_sb[:], va_ps[:], tlenr[:])
            for k in range(3):
                kk = DK[k]
                t_ps = ps3.tile([P, BP], F32, tag="psmall", space="PSUM")
                nc.tensor.transpose(t_ps[:kk, :], va_sb[:, DOF[k]:DOF[k] + kk],
                                    id32_ap)
                nc.vector.tensor_copy(out=vecT_a[:kk, k, :], in_=t_ps[:kk, :])

            # PSUM accumulation semantics: start=True lazily zeroes the whole
            # 2KB bank (each byte is overwritten by its first write after the
            # start). So each PSUM tile gets exactly ONE start (first matmul)
            # and ONE stop (last matmul); disjoint sub-regions accumulate
            # independently in between.
            def lin_mms(vcur, lin_ps, stop=True, with_bias=False):
                for k in (0, 2, 1):
                    kk = DKP[k]  # k=2 padded: lin_w pad columns are zero
                    for kx in range(3):
                        kkx = DK[kx]
                        nc.tensor.matmul(
                            lin_ps[:kk, k, :],
                            lhsT=linw_sb[:kkx, kx, DOF[k]:DOF[k] + kk],
                            rhs=vcur[:kkx, kx, :],
                            start=(k == 0 and kx == 0),
                            stop=(stop and k == 1 and kx == 2))
                        if kx == 0 and with_bias:
                            # bias rides inside the group, never last (the
                            # stop matmul must span 128 partitions)
                            nc.tensor.matmul(
                                lin_ps[:kk, k, :],
                                lhsT=aux16_sb[0:1, A6_LINB + DOF[k]:
                                              A6_LINB + DOF[k] + kk],
                                rhs=onesr_ap[0:1, 0:BP],
                                start=False, stop=False)

            def svec_bc(vcur):
                """svec = vec @ w_vec + attn_b, broadcast to [P, NCH]."""
                svec_ps = ps3.tile([1, BP], F32, tag="psmall", space="PSUM")
                for k in range(3):
                    kk = DK[k]
                    nc.tensor.matmul(svec_ps[:],
                                     lhsT=aux16_sb[:kk, A6_WVEC + k:A6_WVEC + k + 1],
                                     rhs=vcur[:kk, k, :],
                                     start=(k == 0), stop=(k == 2))
                nc.vector.tensor_scalar_add(svec_sb[:], svec_ps[:],
                                            auxf_sb[0:1, AF_ATTNB:AF_ATTNB + 1])
                svbc_ps = ps.tile([P, NCH], F32, tag="svbc", space="PSUM")
                nc.tensor.matmul(svbc_ps[:], lhsT=onesr_ap,
                                 rhs=_row_rep4(svec_sb[:]),
                                 start=True, stop=True)
                return svbc_ps

            def e_mm(e_ps, alpha_col, c, stop_at_end=True):
                """accumulate the next hop's svec attention part:
                E[0, j] += (emb@w_vec)_c^T @ alpha_c"""
                g, cc = divmod(c, GW)
                b = c % BP
                nc.tensor.matmul(e_ps[0:1, b:b + 1],
                                 lhsT=mem_sb[g][:, cc, WV_COL:WV_COL + 1],
                                 rhs=alpha_col,
                                 start=(c == vcols[0]),
                                 stop=(stop_at_end and c == vcols[-1]))

            def attn_mms(attn_ps, alpha_col, c, opened=False):
                """3 accumulating matmuls: attn^T[:, b] += mem_c^T-slices @ alpha.

                The whole [P, 3, BP] tile is one PSUM group per hop: start on
                the very first matmul (or earlier, if the lin matmuls opened
                the group), stop on the very last (see note above).
                """
                g, cc = divmod(c, GW)
                b = c % BP
                # k order (0, 2, 1): the group's first AND last matmul must
                # span all 128 partitions (start/stop flag the bank on the
                # instruction's own partition range only). k=2 is padded to
                # 128 (the pad picks up the packed score columns, but those
                # rows of vec^T are never read).
                for k in (0, 2, 1):
                    kk = DKP[k]
                    nc.tensor.matmul(
                        attn_ps[:kk, k, b:b + 1],
                        lhsT=mem_sb[g][:, cc, DOF[k]:DOF[k] + kk],
                        rhs=alpha_col,
                        start=(not opened and c == vcols[0] and k == 0),
                        stop=(c == vcols[-1] and k == 1))

            def denom_row(e_src):
                """1/denom row [1, BP] via two PE reductions."""
                cs_ps = ps3.tile([P, 1], F32, tag="psmall", space="PSUM")
                nc.tensor.matmul(cs_ps[:], lhsT=e_src, rhs=ones_ap,
                                 start=True, stop=True)
                nc.vector.tensor_copy(out=csc_sb[:], in_=cs_ps[:])
                dn_ps = ps3.tile([1, BP], F32, tag="psmall", space="PSUM")
                nc.tensor.matmul(dn_ps[:], lhsT=csc_sb[:], rhs=gsel_ap,
                                 start=True, stop=True)
                with nc.allow_low_precision(reason="fp16 1/denom, rel 5e-4"):
                    nc.vector.reciprocal(rdr_sb[:], dn_ps[:])

            def denom_bcast():
                """broadcast 1/denom across partitions, staged to SBUF (HW: a
                DVE op may read at most one PSUM operand)."""
                rd_ps = ps3.tile([P, BP], F32, tag="psmall", space="PSUM")
                nc.tensor.matmul(rd_ps[:], lhsT=onesr_ap, rhs=rdr_sb[:],
                                 start=True, stop=True)
                nc.vector.tensor_copy(out=rd_sb[:], in_=rd_ps[:])
                return rd_sb

            def assemble(attn_ps, rd_bc, lin_ps, vnxt):
                # Phase A: the whole (fully-written, padded) attn tile scaled
                # by 1/denom broadcast over the k sections; phase B: add the
                # lin+bias PSUM group. One op each.
                rd3 = bass.AP(rd_bc[:].tensor, rd_bc[:].offset,
                              [list(rd_bc[:].ap[0]), [0, 3], [1, BP]])
                nc.vector.tensor_tensor(out=asm3[:], in0=attn_ps[:],
                                        in1=rd3, op=mult)
                nc.vector.tensor_tensor(out=vnxt[:], in0=lin_ps[:],
                                        in1=asm3[:], op=addop)

            # ======== hop 1 (pipelined per gather group) ========
            lin_ps = ps.tile([P, 3, BP], F32, tag="accL", space="PSUM")
            svbc_ps = svec_bc(vecT_a)
            attn_ps = ps.tile([P, 3, BP], F32, tag="accA", space="PSUM")
            E2_ps = ps.tile([1, BP], F32, tag="Eacc", space="PSUM", bufs=2)
            for g in range(NGRP):
                w = gw[g]
                if w == 0:
                    continue
                gs = g * GW
                score_ap = _ap2d(mem_sb[g][:], SCORE_COL, EPAD16, w)
                nc.vector.tensor_tensor(out=msv[:, gs:gs + w], in0=score_ap,
                                        in1=vloc_ap[:, gs:gs + w], op=mult)
                st = scr.tile([P, GW], F32, tag="st", bufs=4)
                nc.vector.tensor_tensor(out=st[:, 0:w], in0=msv[:, gs:gs + w],
                                        in1=svbc_ps[:, gs:gs + w], op=addop)
                nc.scalar.activation(st[:, 0:w], st[:, 0:w], AF.Tanh)
                nc.scalar.activation(st[:, 0:w], st[:, 0:w], AF.Exp)
                nc.vector.tensor_tensor(out=e_m[:, gs:gs + w], in0=st[:, 0:w],
                                        in1=cmask_ap[:, gs:gs + w], op=mult)
                nc.vector.tensor_tensor(out=aw1[g][:, 0:w], in0=st[:, 0:w],
                                        in1=cv_ap[:, gs:gs + w], op=mult)
                for cc in range(w):
                    attn_mms(attn_ps, aw1[g][:, cc:cc + 1], gs + cc)
                    e_mm(E2_ps, aw1[g][:, cc:cc + 1], gs + cc)
            # emitted after the group streams so the PE does not head-of-line
            # block on the deferred lin_w DMA
            lin_mms(vecT_a, lin_ps, with_bias=True)
            denom_row(e_m[:])
            rd_bc = denom_bcast()
            assemble(attn_ps, rd_bc, lin_ps, vecT_b)

            if DEBUG:
                dbg_va = sb.tile([BP, D], F32, tag="dbg_va_t")
                nc.vector.tensor_copy(out=dbg_va[:], in_=va_sb[:])
                nc.sync.dma_start(dbg_va_d.ap(), dbg_va[:])
                dbg_msv = sb.tile([P, NCH], F32, tag="dbg_msv_t")
                nc.vector.tensor_copy(out=dbg_msv[:], in_=msv[:])
                nc.sync.dma_start(dbg_msv_d.ap(), dbg_msv[:])
                dbg_em = sb.tile([P, NCH], F32, tag="dbg_em_t")
                nc.vector.tensor_copy(out=dbg_em[:], in_=e_m[:])
                nc.sync.dma_start(dbg_em_d.ap(), dbg_em[:])
                dbg_v1 = sb.tile([P, 3, BP], F32, tag="dbg_v1_t")
                nc.vector.memset(dbg_v1[:], 0.0)
                for k in range(3):
                    nc.vector.tensor_copy(out=dbg_v1[:DK[k], k, :],
                                          in_=vecT_b[:DK[k], k, :])
                nc.sync.dma_start(dbg_v1_d.ap(), dbg_v1[:])

            def svbc_mm():
                """broadcast svec_sb across partitions (K=1 matmul)."""
                svbc_ps = ps.tile([P, NCH], F32, tag="svbc", space="PSUM")
                nc.tensor.matmul(svbc_ps[:], lhsT=onesr_ap,
                                 rhs=_row_rep4(svec_sb[:]),
                                 start=True, stop=True)
                return svbc_ps

            def weights_from_svec(svbc_ps):
                """svec broadcast -> pre-scaled alpha weights awn."""
                nc.vector.tensor_tensor(out=sc_f[:], in0=msv[:],
                                        in1=svbc_ps[:], op=addop)
                nc.scalar.activation(sc_f[:], sc_f[:], AF.Tanh)
                nc.scalar.activation(sc_f[:], sc_f[:], AF.Exp)
                nc.vector.tensor_tensor(out=e_m[:], in0=sc_f[:],
                                        in1=cmask_ap, op=mult)
                nc.vector.tensor_tensor(out=aw[:], in0=sc_f[:],
                                        in1=cv_ap, op=mult)
                denom_row(e_m[:])
                rdn_ps = ps.tile([P, NCH], F32, tag="svbc", space="PSUM")
                nc.tensor.matmul(rdn_ps[:], lhsT=onesr_ap,
                                 rhs=_row_rep4(rdr_sb[:]),
                                 start=True, stop=True)
                nc.vector.tensor_tensor(out=awn[:], in0=aw[:],
                                        in1=rdn_ps[:], op=mult)

            # ======== middle hops ========
            # The lin matmuls open the attention PSUM group (attn accumulates
            # on top of vec@lin_w + lin_b); alpha is pre-scaled by the
            # broadcast 1/denom, so the hop result is one PSUM readout.
            # svec comes from the PREVIOUS hop's stream: E = sum_c
            # (emb@w_vec)_c @ alpha_c accumulated alongside it, so the score
            # chain starts without waiting for the assembled vec.
            for h in range(1, N_HOPS - 1):
                vcur = vecT_b if h % 2 == 1 else vecT_a
                vnxt = vecT_a if h % 2 == 1 else vecT_b
                # svec = E2 * (1/denom of hop1) + vec0@(lin_w@w_vec) + const
                lwv_ps = ps3.tile([1, BP], F32, tag="psmall", space="PSUM")
                for k in range(3):
                    kk = DK[k]
                    nc.tensor.matmul(lwv_ps[:],
                                     lhsT=aux16_sb[:kk, A6_LWV + k:A6_LWV + k + 1],
                                     rhs=vecT_a[:kk, k, :],
                                     start=(k == 0), stop=(k == 2))
                sv_t = scr.tile([1, BP], F16, tag="svt", bufs=2)
                nc.vector.tensor_tensor(out=sv_t[:], in0=E2_ps[:],
                                        in1=rdr_sb[:], op=mult)
                nc.vector.scalar_tensor_tensor(
                    out=svec_sb[:], in0=lwv_ps[:],
                    scalar=auxf_sb[0:1, AF_LWVB:AF_LWVB + 1],
                    in1=sv_t[:], op0=addop, op1=addop)
                weights_from_svec(svbc_mm())
                attn_ps = ps.tile([P, 3, BP], F32, tag="accA", space="PSUM")
                lin_mms(vcur, attn_ps, stop=False, with_bias=True)
                E3_ps = ps.tile([1, BP], F32, tag="Eacc", space="PSUM",
                                bufs=2)
                # E3 (next hop's svec) runs BEFORE the attention stream so
                # hop 3's score chain overlaps it; svec3's lin part
                # accumulates into the same E group.
                for c in vcols:
                    e_mm(E3_ps, awn[:, c:c + 1], c, stop_at_end=False)
                for k in range(3):
                    kk = DK[k]
                    nc.tensor.matmul(E3_ps[:],
                                     lhsT=aux16_sb[:kk, A6_LWV + k:A6_LWV + k + 1],
                                     rhs=vcur[:kk, k, :],
                                     start=False, stop=(k == 2))
                # hop3's svec + its partition broadcast go BEFORE the attn
                # stream on the PE, so hop3's score chain is not queued
                # behind it
                nc.vector.tensor_scalar_add(svec_sb[:], E3_ps[:],
                                            auxf_sb[0:1, AF_LWVB:AF_LWVB + 1])
                svbc3_ps = svbc_mm()
                for c in vcols:
                    attn_mms(attn_ps, awn[:, c:c + 1], c, opened=True)
                last_attn = attn_ps

            # ======== final hop fused with the output projection ========
            # logits^T = sum_c (emb@out_w)_c^T @ alpha_c
            #          + (lin_w@out_w)^T @ vec_prev + (lin_b@out_w + out_b)
            # The score chain is emitted BEFORE hop2's PSUM readout so the
            # DVE/ACT chain overlaps hop2's attention stream (only the final
            # awn write has a WAR on the stream's alpha reads).
            weights_from_svec(svbc3_ps)
            vfin = vecT_a if N_HOPS % 2 == 1 else vecT_b
            # readout on the ACT engine so the DVE score chain for the final
            # hop is not blocked behind it (GPSIMD cannot access PSUM)
            nc.scalar.activation(vfin[:], last_attn[:], AF.Copy)
            if DEBUG:
                dbg_v2 = sb.tile([P, 3, BP], F32, tag="dbg_v2_t")
                nc.vector.memset(dbg_v2[:], 0.0)
                for k in range(3):
                    nc.vector.tensor_copy(out=dbg_v2[:DK[k], k, :],
                                          in_=vfin[:DK[k], k, :])
                nc.sync.dma_start(dbg_v2_d.ap(), dbg_v2[:])
            lg_ps = ps3.tile([C, BP], F32, tag="psmall", space="PSUM")
            for k in range(3):
                kk = DK[k]
                nc.tensor.matmul(
                    lg_ps[:],
                    lhsT=aux16_sb[:kk, A6_LWOW + k * C:A6_LWOW + (k + 1) * C],
                    rhs=vfin[:kk, k, :], start=(k == 0), stop=False)
            nc.tensor.matmul(lg_ps[:],
                             lhsT=aux16_sb[0:1, A6_OUTB:A6_OUTB + C],
                             rhs=onesr_ap[0:1, 0:BP], start=False, stop=False)
            for c in vcols:
                g, cc = divmod(c, GW)
                b = c % BP
                nc.tensor.matmul(lg_ps[:, b:b + 1],
                                 lhsT=mem_sb[g][:, cc, OW_COL:OW_COL + C],
                                 rhs=awn[:, c:c + 1],
                                 start=False, stop=(c == vcols[-1]))
            nc.vector.tensor_copy(out=lg_sb[:], in_=lg_ps[:])
            nc.sync.dma_start(out_d.ap(), lg_sb[:])

    nc.compile()
    return nc


def _slot_order(context_len):
    """Slot permutation: batches sorted by descending context_len."""
    return np.argsort(-np.asarray(context_len), kind="stable")


def _band_counts(context_len):
    """(m1, m2, m3): #batches with len > 128r for r = 1, 2, 3."""
    cl = np.asarray(context_len)
    return tuple(int((cl > 128 * r).sum()) for r in (1, 2, 3))


def _wrap16(flat):
    """dma_gather index layout: [128, n/16], replicated over 16-row groups."""
    n = flat.shape[0]
    w = flat.reshape(n // 16, 16).T.astype(np.int16)   # [16, n/16]
    return np.ascontiguousarray(np.tile(w, (8, 1)))    # [128, n/16]


def make_core_inputs(context_x, context_len, target_x, target_len, target_loc,
                     emb16, shared):
    """Per-core input dict. context_x etc are the 32-row shards (numpy).

    The embedding table is sharded per core by index compaction: each core
    receives only the (unique) rows its context references, packed as 768B
    rows (300 fp8 values + the precomputed fp16 content score emb@w_mem at
    byte 300), plus int16 local indices in the wrapped dma_gather layout.
    Target rows (160 per core) are materialized host-side in fp16.
    """
    score16 = shared["_score16"]
    order = _slot_order(context_len)
    # flat gather stream in (c, p) order: chunk c = 32r + j holds
    # l in [128r, 128r+128) of batch order[j]
    cj = np.arange(NCH) % BP
    cr = np.arange(NCH) // BP
    bmap = order[cj]                                       # [NCH]
    flat = np.zeros((NCH, P), np.int64)
    for c in range(NCH):
        flat[c] = context_x[bmap[c], cr[c] * P:(cr[c] + 1) * P]
    flat = flat.reshape(-1)
    uniq, inv = np.unique(flat, return_inverse=True)
    assert uniq.shape[0] <= U_PAD
    emb_loc = np.zeros((U_PAD, EPAD16), np.float16)
    emb_loc[:uniq.shape[0], :D] = emb16[uniq]
    emb_loc[:uniq.shape[0], SCORE_COL] = score16[uniq]
    emb_loc[:uniq.shape[0], OW_COL:OW_COL + C] = shared["_ow16"][uniq]
    emb_loc[:uniq.shape[0], WV_COL] = shared["_wv16"][uniq]
    ctx_idx = _wrap16(inv)
    nspl = 4 * GW * P // 16

    # host-materialized target rows: row r = j*128 + p -> t = r//32, b = r%32
    tgtr = np.zeros((P, TCOL, TE), np.float16)
    for j in range(TCOL):
        for t0 in range(P // BP):
            t = j * (P // BP) + t0
            if t >= T:
                break
            rows = emb16[target_x[:, t]]                 # [BP, D]
            tgtr[t0 * BP:(t0 + 1) * BP, j, :D] = rows
    tgtr = tgtr.reshape(P, TCOL * TE)

    # host-side location model per (p, c): c = 32r + j -> b = order[j],
    # l = 128r + p
    pos = (cr[None, :] * P + np.arange(P)[:, None]).astype(np.float64)
    loc_b = target_loc[bmap].astype(np.float64)[None, :]
    len_b = context_len[bmap].astype(np.float64)[None, :]
    vloc = 1.0 - np.abs(pos - loc_b) / len_b
    cmask = (pos < len_b).astype(np.float64)

    auxf = np.zeros((P, AF_N), np.float32)
    auxf[:, AF_TLEN] = target_len[np.arange(P) % BP]
    auxf[:, AF_T0] = np.arange(P) // BP
    auxf[:, AF_T1] = (P // BP) + np.arange(P) // BP
    auxf[:BP, AF_ID32:AF_ID32 + BP] = np.eye(BP)
    auxf[:C, AF_OUTB] = shared["_outb"]
    auxf[0, AF_ATTNB] = shared["_attnb"]
    auxf[:BP, AF_TLENP] = target_len[order]
    auxf[:, AF_SSEL:AF_SSEL + BP] = (
        np.arange(P)[:, None] % BP == order[None, :])
    auxf[0, AF_LWVB] = shared["_lwvb"]

    aux16 = shared["aux16_h"].copy()
    aux16[:, A6_VLOC:A6_VLOC + NCH] = vloc
    aux16[:, A6_CMASK:A6_CMASK + NCH] = cmask
    aux16[:, A6_CV:A6_CV + NCH] = cmask * vloc

    d = dict(aux16_h=aux16, lin_w_h=shared["lin_w_h"])
    d.update(emb_loc=emb_loc,
             ctx_idx16a=np.ascontiguousarray(ctx_idx[:, :nspl]),
             ctx_idx16b=np.ascontiguousarray(ctx_idx[:, nspl:]),
             tgtr_h=tgtr, auxf_h=auxf)
    return d


def make_shared_inputs(emb, attn_w, attn_b, lin_w, lin_b, out_w, out_b):
    lin_w_pad = np.zeros((384, 384), np.float16)
    lin_w_pad[:D, :D] = lin_w.astype(np.float16)
    lin_w_h = np.ascontiguousarray(
        lin_w_pad.reshape(3, P, 384).transpose(1, 0, 2).reshape(P, 3 * 384))

    aux16 = np.zeros((P, A6_N), np.float16)
    # gsel: chunk (partition) c -> slot column c % 32
    aux16[:, A6_GSEL:A6_GSEL + BP] = (
        np.arange(P)[:, None] % BP == np.arange(BP)[None, :])
    w_vec_pad = np.zeros((384,), np.float16)
    w_vec_pad[:D] = attn_w[D:, 0].astype(np.float16)
    aux16[:, A6_WVEC:A6_WVEC + 3] = w_vec_pad.reshape(3, P).T
    aux16[:, A6_ONES] = 1.0
    aux16[0, A6_ONESR:A6_ONESR + P] = 1.0
    out_w_pad = np.zeros((384, C), np.float16)
    out_w_pad[:D] = out_w.astype(np.float16)
    aux16[:, A6_OUTW:A6_OUTW + 3 * C] = (
        out_w_pad.reshape(3, P, C).transpose(1, 0, 2).reshape(P, 3 * C))
    aux16[0, A6_LINB:A6_LINB + D] = lin_b.astype(np.float16)
    aux16[0, A6_OUTB:A6_OUTB + C] = (
        np.asarray(out_b, np.float64)
        + np.asarray(lin_b, np.float64) @ np.asarray(out_w, np.float64)
    ).astype(np.float16)
    lwow = np.asarray(lin_w, np.float64) @ np.asarray(out_w, np.float64)
    lwow_pad = np.zeros((384, C), np.float16)
    lwow_pad[:D] = lwow.astype(np.float16)
    aux16[:, A6_LWOW:A6_LWOW + 3 * C] = (
        lwow_pad.reshape(3, P, C).transpose(1, 0, 2).reshape(P, 3 * C))
    lwv = np.asarray(lin_w, np.float64) @ np.asarray(attn_w[D:, 0], np.float64)
    lwv_pad = np.zeros((384,), np.float16)
    lwv_pad[:D] = lwv.astype(np.float16)
    aux16[:, A6_LWV:A6_LWV + 3] = lwv_pad.reshape(3, P).T

    lin_b_pad = np.zeros((384,), np.float32)
    lin_b_pad[:D] = lin_b
    score16 = (np.asarray(emb, np.float64)
               @ np.asarray(attn_w[:D, 0], np.float64)).astype(np.float16)
    ow16 = (np.asarray(emb, np.float64)
            @ np.asarray(out_w, np.float64)).astype(np.float16)
    wv16 = (np.asarray(emb, np.float64)
            @ np.asarray(attn_w[D:, 0], np.float64)).astype(np.float16)
    lwvb = np.float32(np.asarray(lin_b, np.float64)
                      @ np.asarray(attn_w[D:, 0], np.float64)
                      + np.float64(attn_b[0]))
    outb_eff = (np.asarray(out_b, np.float64)
                + np.asarray(lin_b, np.float64)
                @ np.asarray(out_w, np.float64)).astype(np.float32)
    return dict(
        lin_w_h=lin_w_h,
        aux16_h=aux16,
        _linb3=np.ascontiguousarray(lin_b_pad.reshape(3, P).T),
        _outb=outb_eff,
        _attnb=np.float32(attn_b[0]),
        _score16=score16,
        _ow16=ow16,
        _wv16=wv16,
        _lwvb=lwvb,
    )


_module_cache = {}


def get_module(m=None):
    if m is None:
        # most-recently built module (test.py convenience)
        return next(reversed(_module_cache.values()))
    if m not in _module_cache:
        _module_cache[m] = build_module(m)
    return _module_cache[m]


def kernel(**inputs):
    emb16 = np.ascontiguousarray(inputs["emb"].astype(np.float16))
    shared = make_shared_inputs(
        np.asarray(inputs["emb"]), np.asarray(inputs["attn_w"]),
        np.asarray(inputs["attn_b"]), np.asarray(inputs["lin_w"]),
        np.asarray(inputs["lin_b"]), np.asarray(inputs["out_w"]),
        np.asarray(inputs["out_b"]))
    context_len = np.asarray(inputs["context_len"])
    in_maps = []
    orders = []
    mm = (0, 0, 0)
    for k in range(N_CORES):
        s = slice(k * BP, (k + 1) * BP)
        in_maps.append(make_core_inputs(
            np.asarray(inputs["context_x"])[s],
            context_len[s],
            np.asarray(inputs["target_x"])[s],
            np.asarray(inputs["target_len"])[s],
            np.asarray(inputs["target_loc"])[s],
            emb16, shared))
        orders.append(_slot_order(context_len[s]))
        mm = tuple(max(a, b) for a, b in
                   zip(mm, _band_counts(context_len[s])))
    nc = get_module(mm)
    res = bass_utils.run_bass_kernel_spmd(nc, in_maps,
                                          core_ids=list(range(N_CORES)))
    out = np.empty((B, C), np.float32)
    for k in range(N_CORES):
        blk = res.results[k]["logits_t"].T.astype(np.float32)  # [slot, C]
        out[k * BP + orders[k]] = blk
    return out


# revision 137
# speedup vs baseline: 1.0045x; 1.0045x over previous
"""MemNet (scatter_memory) Trainium2 kernel, v2.

Model (per batch row b):
  memory   = emb[context_x[b]]                    # [L, D] gather
  v_aspect = masked-mean(emb[target_x[b]])        # [D]
  v_loc    = 1 - |pos - target_loc[b]| / context_len[b]
  3 hops of: scores = tanh((memory*v_loc) @ w_mem + vec@w_vec + b)
             alpha  = masked softmax;  vec = alpha @ (memory*v_loc) + vec@lin_w+lin_b
  logits   = vec @ out_w + out_b

Sharding: data-parallel over batch, 32 rows per core on 8 cores; the fp16
embedding table is index-compacted per core and fetched by indirect DMA
gather.

Key structure (vs v1):
- The content score emb@w_mem is a pure weight transform; it is appended
  as column 300 of the gathered rows (inside the 768B alignment padding),
  so scores arrive with the gather for free.
- Attention contraction runs transposed on the PE: per (chunk, d-slice),
  stationary = mem[128, DK], moving = the chunk's alpha column [128, 1],
  accumulating over each batch row's 4 chunks directly into vec^T layout
  [DK, b] in PSUM. Cost ~ 1 cycle per matmul (output free size 1).
- v_loc/cmask are host-side index-derived tensors; hop 1 (whose alpha
  depends only on v_aspect) is pipelined per gather group so its attention
  runs under the gather phase; the softmax denominator accumulates per
  group in an open PSUM matmul group.
"""

import numpy as np

import concourse.bass as bass
import concourse.bacc as bacc
import concourse.mybir as mybir
import concourse.tile as tile
from concourse import bass_utils

N_CORES = 8
B, L, T, V, D, C = 256, 512, 5, 50000, 300, 3
N_HOPS = 3
BP = B // N_CORES          # 32 batch rows per core
P = 128                    # partitions
NCH = (BP * L) // P        # 128 chunk columns; chunk c: b=c//4, l=(c%4)*128+p
CPB = L // P               # 4 chunks per batch row
NGRP = 16                  # gather groups (<=1024 idxs per dma_gather)
GW = NCH // NGRP           # chunk columns per gather group (8)
DK = [128, 128, 44]        # D split across PSUM partition chunks
DKP = [128, 128, 128]      # k=2 padded to 128 (pad rows are dead/zero) so
                           # PSUM tiles are fully written -> one big readout
DOF = [0, 128, 256]
TCOL = (BP * T + P - 1) // P  # 2 columns of host-provided target rows
EPAD16 = 384               # row length in f16 units (768B rows: 300 fp16
                           # values + packed per-row weight transforms)
SCORE_COL = 300            # f16 column of the packed content score emb@w_mem
OW_COL = 304               # [304:307) packed output scores emb@out_w
WV_COL = 307               # packed emb@w_vec (feeds the next hop's svec)
TE = 304                   # target row pad (f16 units)
U_PAD = 16768              # fixed local-table rows (>= 16384)

F16 = mybir.dt.float16
I16 = mybir.dt.int16
F32 = mybir.dt.float32
F8 = mybir.dt.float8e4

# auxf (f32) column layout
AF_TLEN = 0            # target_len per partition (p % 32)
AF_T0 = 1              # target t-index for j=0 (p // 32)
AF_T1 = 2              # target t-index for j=1 (4 + p // 32)
AF_ID32 = 3            # [3:35) id32 (rows 0:32)
AF_OUTB = 35           # effective out bias (rows 0:3)
AF_ATTNB = 36          # attn_b (row 0)
AF_TLENP = 37          # target_len per slot (rows 0:32)
AF_SSEL = 38           # [38:70) ssel: target row p -> slot column
AF_LWVB = 70           # lin_b @ w_vec + attn_b (row 0)
AF_N = 71

# aux16 (f16) column layout
A6_GSEL = 0            # [0:32)   gsel: chunk c (partition) -> b
A6_SSEL = 32           # [32:64)  ssel: target row p -> b = p % 32
A6_WVEC = 64           # [64:67)  w_vec d-chunks
A6_ONES = 67           # ones column
A6_ONESR = 68          # [68:196) ones row (partition 0)
A6_OUTW = 196          # [196:205) out_w  [d-part, 3 k, C]
A6_LINB = 205          # [205:589) lin_b as an f16 row (partition 0)
A6_LWOW = 589          # [589:598) lin_w @ out_w  [d-part, 3 k, C]
A6_VLOC = 598          # [598:726) vloc
A6_CMASK = 726         # [726:854) cmask
A6_CV = 854            # [854:982) cmask * vloc
A6_OUTB = 982          # [982:985) effective out bias row (partition 0)
A6_LWV = 985           # [985:988) lin_w @ w_vec d-chunks
A6_N = 988


def _ap2d(tile_ap, col_off, stride, n):
    """2D AP over a 3D tile: partition dim + one strided free dim."""
    return bass.AP(tile_ap.tensor, tile_ap.offset + col_off,
                   [list(tile_ap.ap[0]), [stride, n]])


def _row_rep4(ap2):
    """[1, BP] row AP -> [1, CPB, BP] with the outer dim broadcast (step 0),
    so column c = 32*r + j reads value j."""
    return bass.AP(ap2.tensor, ap2.offset,
                   [list(ap2.ap[0]), [0, CPB], [1, BP]])


DEBUG = False


def build_module(m=(BP, BP, BP)):
    """m = (m1, m2, m3): valid slot count per l-band r=1..3 (band 0 is always
    full). Chunk column c = 32*r + j holds l in [128r, 128r+128) of the
    batch in slot j (host sorts batches by descending context_len, so valid
    chunks are a prefix of each band); only valid chunks are gathered.
    """
    m_band = [BP, m[0], m[1], m[2]]
    # per-gather-group valid widths (group g = columns [8g, 8g+8))
    gw = [max(0, min(GW, m_band[(8 * g) // BP] - (8 * g) % BP))
          for g in range(NGRP)]
    vcols = [8 * g + cc for g in range(NGRP) for cc in range(gw[g])]

    nc = bacc.Bacc("TRN2", target_bir_lowering=False, debug=False,
                   num_devices=N_CORES)

    emb_d = nc.dram_tensor("emb_loc", [U_PAD, EPAD16], F16,
                           kind="ExternalInput")
    NIC = GW * P // 16  # idx tile columns per full group
    NSPL = 4 * NIC      # groups 0..3 arrive in a small first DMA
    ctx_idxa_d = nc.dram_tensor("ctx_idx16a", [P, NSPL], I16,
                                kind="ExternalInput")
    ctx_idxb_d = nc.dram_tensor("ctx_idx16b", [P, NCH * P // 16 - NSPL], I16,
                                kind="ExternalInput")
    auxf_d = nc.dram_tensor("auxf_h", [P, AF_N], F32, kind="ExternalInput")
    # target rows ride at the tail of aux16 (one fewer HWDGE slot at startup)
    aux16_d = nc.dram_tensor("aux16_h", [P, A6_N + TCOL * TE], F16,
                             kind="ExternalInput")
    linw_d = nc.dram_tensor("lin_w_h", [P, 3 * 384], F16, kind="ExternalInput")

    out_d = nc.dram_tensor("logits_t", [C, BP], F32, kind="ExternalOutput")
    if DEBUG:
        dbg_msv_d = nc.dram_tensor("dbg_msv", [P, NCH], F32,
                                   kind="ExternalOutput")
        dbg_em_d = nc.dram_tensor("dbg_em", [P, NCH], F32,
                                  kind="ExternalOutput")
        dbg_va_d = nc.dram_tensor("dbg_va", [BP, D], F32,
                                  kind="ExternalOutput")
        dbg_v1_d = nc.dram_tensor("dbg_v1", [P, 3 * BP], F32,
                                  kind="ExternalOutput")
        dbg_v2_d = nc.dram_tensor("dbg_v2", [P, 3 * BP], F32,
                                  kind="ExternalOutput")

    mult = mybir.AluOpType.mult
    addop = mybir.AluOpType.add
    is_lt = mybir.AluOpType.is_lt
    AF = mybir.ActivationFunctionType

    with tile.TileContext(nc) as tc:
        with (
            tc.tile_pool(name="sb", bufs=1) as sb,
            tc.tile_pool(name="sc", bufs=4) as scr,
            tc.tile_pool(name="ps", bufs=1, space="PSUM") as ps,
            tc.tile_pool(name="ps3", bufs=2, space="PSUM") as ps3,
        ):
            # ---- persistent SBUF tiles ----
            idxa_sb = sb.tile([P, NSPL], I16, tag="idxa")
            idxb_sb = sb.tile([P, NCH * P // 16 - NSPL], I16, tag="idxb")
            auxf_sb = sb.tile([P, AF_N], F32, tag="auxf")
            aux16_sb = sb.tile([P, A6_N + TCOL * TE], F16, tag="aux16")
            linw_sb = sb.tile([P, 3, 384], F16, tag="linw")
            mem_sb = [sb.tile([P, GW, EPAD16], F16, tag=f"mem{g}",
                              name=f"mem{g}") for g in range(NGRP)]

            tmask = sb.tile([P, TCOL], F32, tag="tmask")
            a0 = sb.tile([P, BP, TCOL], F16, tag="a0")
            tlenr = sb.tile([BP, 1], F32, tag="tlenr")
            va_sb = sb.tile([BP, D], F32, tag="va")
            vecT_a = sb.tile([P, 3, BP], F16, tag="vecTa", name="vecT_a")
            vecT_b = sb.tile([P, 3, BP], F16, tag="vecTb", name="vecT_b")
            msv = sb.tile([P, NCH], F32, tag="msv")
            sc_f = sb.tile([P, NCH], F32, tag="scf")
            e_m = sb.tile([P, NCH], F16, tag="em")
            aw = sb.tile([P, NCH], F16, tag="aw")
            awn = sb.tile([P, NCH], F16, tag="awn")
            aw1 = [sb.tile([P, GW], F16, tag=f"aw1_{g}", name=f"aw1_{g}")
                   for g in range(NGRP)]
            svec_sb = sb.tile([1, BP], F16, tag="svec")
            csc_sb = sb.tile([P, 1], F16, tag="cs")
            rdr_sb = sb.tile([1, BP], F16, tag="rdr")
            rd_sb = sb.tile([P, BP], F32, tag="rdbc_s")
            asm3 = sb.tile([P, 3, BP], F32, tag="asm")
            lg_sb = sb.tile([C, BP], F32, tag="lg")

            vloc_ap = aux16_sb[:, A6_VLOC:A6_VLOC + NCH]
            cmask_ap = aux16_sb[:, A6_CMASK:A6_CMASK + NCH]
            cv_ap = aux16_sb[:, A6_CV:A6_CV + NCH]
            gsel_ap = aux16_sb[:, A6_GSEL:A6_GSEL + BP]
            ssel_ap = auxf_sb[:, AF_SSEL:AF_SSEL + BP]
            ones_ap = aux16_sb[:, A6_ONES:A6_ONES + 1]
            onesr_ap = aux16_sb[0:1, A6_ONESR:A6_ONESR + P]
            id32_ap = auxf_sb[0:BP, AF_ID32:AF_ID32 + BP]

            # ---- input DMAs (context indices first so gathers start early) ----
            nc.sync.dma_start(idxa_sb[:], ctx_idxa_d.ap())
            nc.sync.dma_start(idxb_sb[:], ctx_idxb_d.ap())
            nc.sync.dma_start(aux16_sb[:], aux16_d.ap())
            nc.sync.dma_start(auxf_sb[:], auxf_d.ap())

            # uninvolved score slots must stay finite: zero msv/e_m once so
            # chunks never gathered (invalid/pad) read as 0 through the
            # softmax (cmask/cv are 0 there host-side)
            nc.vector.memset(msv[:], 0.0)
            nc.vector.memset(e_m[:], 0.0)

            # ---- gathers (768B rows: fp16 values + fp16 score), only the
            # valid prefix of each group ----
            for g in range(NGRP):
                if gw[g] == 0:
                    continue
                nig = gw[g] * P
                if g < 4:
                    iap = idxa_sb[:, g * NIC:g * NIC + nig // 16]
                else:
                    g4 = g - 4
                    iap = idxb_sb[:, g4 * NIC:g4 * NIC + nig // 16]
                nc.gpsimd.dma_gather(
                    out_ap=mem_sb[g][:, 0:gw[g], :], in_ap=emb_d.ap(),
                    idxs_ap=iap, num_idxs=nig, num_idxs_reg=nig,
                    elem_size=EPAD16)

            # lin_w (294KB) is only needed for hop-1's assembly (~31us), but
            # its DMA would steal gather-rail time. Gate it behind the last
            # gather with a 1-element copy (WAW on linw_sb) so all gathers
            # finish ~1.3us earlier.
            glast = max(g for g in range(NGRP) if gw[g] > 0)
            nc.gpsimd.tensor_copy(out=linw_sb[0:1, 0, 0:1],
                                  in_=mem_sb[glast][0:1, 0, 0:1])
            nc.sync.dma_start(linw_sb[:], linw_d.ap())

            # ---- v_aspect -> vecT_a ----
            nc.vector.tensor_tensor(
                out=tmask[:], in0=auxf_sb[:, AF_T0:AF_T0 + TCOL],
                in1=auxf_sb[:, AF_TLEN:AF_TLEN + 1].to_broadcast([P, TCOL]),
                op=is_lt)
            va_ps = ps.tile([BP, D], F32, tag="acc300", space="PSUM")
            for j in range(TCOL):
                nc.vector.tensor_scalar_mul(a0[:, :, j], ssel_ap,
                                            tmask[:, j:j + 1])
                nc.tensor.matmul(va_ps[:], lhsT=a0[:, :, j],
                                 rhs=_ap2d(aux16_sb[:], A6_N + j * TE, 1, D),
                                 start=(j == 0), stop=(j == TCOL - 1))
            nc.vector.reciprocal(tlenr[:], auxf_sb[0:BP, AF_TLENP:AF_TLENP + 1])
            nc.vector.tensor_scalar_mul(va_sb[:], va_ps[:], tlenr[:])
            for k in range(3):
                kk = DK[k]
                t_ps = ps3.tile([P, BP], F32, tag="psmall", space="PSUM")
                nc.tensor.transpose(t_ps[:kk, :], va_sb[:, DOF[k]:DOF[k] + kk],
                                    id32_ap)
                nc.vector.tensor_copy(out=vecT_a[:kk, k, :], in_=t_ps[:kk, :])

            # PSUM accumulation semantics: start=True lazily zeroes the whole
            # 2KB bank (each byte is overwritten by its first write after the
            # start). So each PSUM tile gets exactly ONE start (first matmul)
            # and ONE stop (last matmul); disjoint sub-regions accumulate
            # independently in between.
            def lin_mms(vcur, lin_ps, stop=True, with_bias=False):
                for k in (0, 2, 1):
                    kk = DKP[k]  # k=2 padded: lin_w pad columns are zero
                    for kx in range(3):
                        kkx = DK[kx]
                        nc.tensor.matmul(
                            lin_ps[:kk, k, :],
                            lhsT=linw_sb[:kkx, kx, DOF[k]:DOF[k] + kk],
                            rhs=vcur[:kkx, kx, :],
                            start=(k == 0 and kx == 0),
                            stop=(stop and k == 1 and kx == 2))
                        if kx == 0 and with_bias:
                            # bias rides inside the group, never last (the
                            # stop matmul must span 128 partitions)
                            nc.tensor.matmul(
                                lin_ps[:kk, k, :],
                                lhsT=aux16_sb[0:1, A6_LINB + DOF[k]:
                                              A6_LINB + DOF[k] + kk],
                                rhs=onesr_ap[0:1, 0:BP],
                                start=False, stop=False)

            def svec_bc(vcur):
                """svec = vec @ w_vec + attn_b, broadcast to [P, NCH]."""
                svec_ps = ps3.tile([1, BP], F32, tag="psmall", space="PSUM")
                for k in range(3):
                    kk = DK[k]
                    nc.tensor.matmul(svec_ps[:],
                                     lhsT=aux16_sb[:kk, A6_WVEC + k:A6_WVEC + k + 1],
                                     rhs=vcur[:kk, k, :],
                                     start=(k == 0), stop=(k == 2))
                nc.vector.tensor_scalar_add(svec_sb[:], svec_ps[:],
                                            auxf_sb[0:1, AF_ATTNB:AF_ATTNB + 1])
                svbc_ps = ps.tile([P, NCH], F32, tag="svbc", space="PSUM")
                nc.tensor.matmul(svbc_ps[:], lhsT=onesr_ap,
                                 rhs=_row_rep4(svec_sb[:]),
                                 start=True, stop=True)
                return svbc_ps

            def e_mm(e_ps, alpha_col, c, stop_at_end=True):
                """accumulate the next hop's svec attention part:
                E[0, j] += (emb@w_vec)_c^T @ alpha_c"""
                g, cc = divmod(c, GW)
                b = c % BP
                nc.tensor.matmul(e_ps[0:1, b:b + 1],
                                 lhsT=mem_sb[g][:, cc, WV_COL:WV_COL + 1],
                                 rhs=alpha_col,
                                 start=(c == vcols[0]),
                                 stop=(stop_at_end and c == vcols[-1]))

            def attn_mms(attn_ps, alpha_col, c, opened=False):
                """3 accumulating matmuls: attn^T[:, b] += mem_c^T-slices @ alpha.

                The whole [P, 3, BP] tile is one PSUM group per hop: start on
                the very first matmul (or earlier, if the lin matmuls opened
                the group), stop on the very last (see note above).
                """
                g, cc = divmod(c, GW)
                b = c % BP
                # k order (0, 2, 1): the group's first AND last matmul must
                # span all 128 partitions (start/stop flag the bank on the
                # instruction's own partition range only). k=2 is padded to
                # 128 (the pad picks up the packed score columns, but those
                # rows of vec^T are never read).
                for k in (0, 2, 1):
                    kk = DKP[k]
                    nc.tensor.matmul(
                        attn_ps[:kk, k, b:b + 1],
                        lhsT=mem_sb[g][:, cc, DOF[k]:DOF[k] + kk],
                        rhs=alpha_col,
                        start=(not opened and c == vcols[0] and k == 0),
                        stop=(c == vcols[-1] and k == 1))

            def denom_row(e_src):
                """1/denom row [1, BP] via two PE reductions."""
                cs_ps = ps3.tile([P, 1], F32, tag="psmall", space="PSUM")
                nc.tensor.matmul(cs_ps[:], lhsT=e_src, rhs=ones_ap,
                                 start=True, stop=True)
                nc.vector.tensor_copy(out=csc_sb[:], in_=cs_ps[:])
                dn_ps = ps3.tile([1, BP], F32, tag="psmall", space="PSUM")
                nc.tensor.matmul(dn_ps[:], lhsT=csc_sb[:], rhs=gsel_ap,
                                 start=True, stop=True)
                with nc.allow_low_precision(reason="fp16 1/denom, rel 5e-4"):
                    nc.vector.reciprocal(rdr_sb[:], dn_ps[:])

            def denom_bcast():
                """broadcast 1/denom across partitions, staged to SBUF (HW: a
                DVE op may read at most one PSUM operand)."""
                rd_ps = ps3.tile([P, BP], F32, tag="psmall", space="PSUM")
                nc.tensor.matmul(rd_ps[:], lhsT=onesr_ap, rhs=rdr_sb[:],
                                 start=True, stop=True)
                nc.vector.tensor_copy(out=rd_sb[:], in_=rd_ps[:])
                return rd_sb

            def assemble(attn_ps, rd_bc, lin_ps, vnxt):
                # Phase A: the whole (fully-written, padded) attn tile scaled
                # by 1/denom broadcast over the k sections; phase B: add the
                # lin+bias PSUM group. One op each.
                rd3 = bass.AP(rd_bc[:].tensor, rd_bc[:].offset,
                              [list(rd_bc[:].ap[0]), [0, 3], [1, BP]])
                nc.vector.tensor_tensor(out=asm3[:], in0=attn_ps[:],
                                        in1=rd3, op=mult)
                nc.vector.tensor_tensor(out=vnxt[:], in0=lin_ps[:],
                                        in1=asm3[:], op=addop)

            # ======== hop 1 (pipelined per gather group) ========
            lin_ps = ps.tile([P, 3, BP], F32, tag="accL", space="PSUM")
            svbc_ps = svec_bc(vecT_a)
            attn_ps = ps.tile([P, 3, BP], F32, tag="accA", space="PSUM")
            E2_ps = ps.tile([1, BP], F32, tag="Eacc", space="PSUM", bufs=2)
            for g in range(NGRP):
                w = gw[g]
                if w == 0:
                    continue
                gs = g * GW
                score_ap = _ap2d(mem_sb[g][:], SCORE_COL, EPAD16, w)
                nc.vector.tensor_tensor(out=msv[:, gs:gs + w], in0=score_ap,
                                        in1=vloc_ap[:, gs:gs + w], op=mult)
                st = scr.tile([P, GW], F32, tag="st", bufs=4)
                nc.vector.tensor_tensor(out=st[:, 0:w], in0=msv[:, gs:gs + w],
                                        in1=svbc_ps[:, gs:gs + w], op=addop)
                nc.scalar.activation(st[:, 0:w], st[:, 0:w], AF.Tanh)
                nc.scalar.activation(st[:, 0:w], st[:, 0:w], AF.Exp)
                nc.vector.tensor_tensor(out=e_m[:, gs:gs + w], in0=st[:, 0:w],
                                        in1=cmask_ap[:, gs:gs + w], op=mult)
                nc.vector.tensor_tensor(out=aw1[g][:, 0:w], in0=st[:, 0:w],
                                        in1=cv_ap[:, gs:gs + w], op=mult)
                for cc in range(w):
                    attn_mms(attn_ps, aw1[g][:, cc:cc + 1], gs + cc)
                    e_mm(E2_ps, aw1[g][:, cc:cc + 1], gs + cc)
            # emitted after the group streams so the PE does not head-of-line
            # block on the deferred lin_w DMA
            lin_mms(vecT_a, lin_ps, with_bias=True)
            denom_row(e_m[:])
            rd_bc = denom_bcast()
            assemble(attn_ps, rd_bc, lin_ps, vecT_b)

            if DEBUG:
                dbg_va = sb.tile([BP, D], F32, tag="dbg_va_t")
                nc.vector.tensor_copy(out=dbg_va[:], in_=va_sb[:])
                nc.sync.dma_start(dbg_va_d.ap(), dbg_va[:])
                dbg_msv = sb.tile([P, NCH], F32, tag="dbg_msv_t")
                nc.vector.tensor_copy(out=dbg_msv[:], in_=msv[:])
                nc.sync.dma_start(dbg_msv_d.ap(), dbg_msv[:])
                dbg_em = sb.tile([P, NCH], F32, tag="dbg_em_t")
                nc.vector.tensor_copy(out=dbg_em[:], in_=e_m[:])
                nc.sync.dma_start(dbg_em_d.ap(), dbg_em[:])
                dbg_v1 = sb.tile([P, 3, BP], F32, tag="dbg_v1_t")
                nc.vector.memset(dbg_v1[:], 0.0)
                for k in range(3):
                    nc.vector.tensor_copy(out=dbg_v1[:DK[k], k, :],
                                          in_=vecT_b[:DK[k], k, :])
                nc.sync.dma_start(dbg_v1_d.ap(), dbg_v1[:])

            def svbc_mm():
                """broadcast svec_sb across partitions (K=1 matmul)."""
                svbc_ps = ps.tile([P, NCH], F32, tag="svbc", space="PSUM")
                nc.tensor.matmul(svbc_ps[:], lhsT=onesr_ap,
                                 rhs=_row_rep4(svec_sb[:]),
                                 start=True, stop=True)
                return svbc_ps

            def weights_from_svec(svbc_ps):
                """svec broadcast -> pre-scaled alpha weights awn."""
                nc.vector.tensor_tensor(out=sc_f[:], in0=msv[:],
                                        in1=svbc_ps[:], op=addop)
                nc.scalar.activation(sc_f[:], sc_f[:], AF.Tanh)
                nc.scalar.activation(sc_f[:], sc_f[:], AF.Exp)
                nc.vector.tensor_tensor(out=e_m[:], in0=sc_f[:],
                                        in1=cmask_ap, op=mult)
                nc.vector.tensor_tensor(out=aw[:], in0=sc_f[:],
                                        in1=cv_ap, op=mult)
                denom_row(e_m[:])
                rdn_ps = ps.tile([P, NCH], F32, tag="svbc", space="PSUM")
                nc.tensor.matmul(rdn_ps[:], lhsT=onesr_ap,
                                 rhs=_row_rep4(rdr_sb[:]),
                                 start=True, stop=True)
                nc.vector.tensor_tensor(out=awn[:], in0=aw[:],
                                        in1=rdn_ps[:], op=mult)

            # ======== middle hops ========
            # The lin matmuls open the attention PSUM group (attn accumulates
            # on top of vec@lin_w + lin_b); alpha is pre-scaled by the
            # broadcast 1/denom, so the hop result is one PSUM readout.
            # svec comes from the PREVIOUS hop's stream: E = sum_c
            # (emb@w_vec)_c @ alpha_c accumulated alongside it, so the score
            # chain starts without waiting for the assembled vec.
            for h in range(1, N_HOPS - 1):
                vcur = vecT_b if h % 2 == 1 else vecT_a
                vnxt = vecT_a if h % 2 == 1 else vecT_b
                # svec = E2 * (1/denom of hop1) + vec0@(lin_w@w_vec) + const
                lwv_ps = ps3.tile([1, BP], F32, tag="psmall", space="PSUM")
                for k in range(3):
                    kk = DK[k]
                    nc.tensor.matmul(lwv_ps[:],
                                     lhsT=aux16_sb[:kk, A6_LWV + k:A6_LWV + k + 1],
                                     rhs=vecT_a[:kk, k, :],
                                     start=(k == 0), stop=(k == 2))
                sv_t = scr.tile([1, BP], F16, tag="svt", bufs=2)
                nc.vector.tensor_tensor(out=sv_t[:], in0=E2_ps[:],
                                        in1=rdr_sb[:], op=mult)
                nc.vector.scalar_tensor_tensor(
                    out=svec_sb[:], in0=lwv_ps[:],
                    scalar=auxf_sb[0:1, AF_LWVB:AF_LWVB + 1],
                    in1=sv_t[:], op0=addop, op1=addop)
                weights_from_svec(svbc_mm())
                attn_ps = ps.tile([P, 3, BP], F32, tag="accA", space="PSUM")
                lin_mms(vcur, attn_ps, stop=False, with_bias=True)
                E3_ps = ps.tile([1, BP], F32, tag="Eacc", space="PSUM",
                                bufs=2)
                # E3 (next hop's svec) runs BEFORE the attention stream so
                # hop 3's score chain overlaps it; svec3's lin part
                # accumulates into the same E group.
                for c in vcols:
                    e_mm(E3_ps, awn[:, c:c + 1], c, stop_at_end=False)
                for k in range(3):
                    kk = DK[k]
                    nc.tensor.matmul(E3_ps[:],
                                     lhsT=aux16_sb[:kk, A6_LWV + k:A6_LWV + k + 1],
                                     rhs=vcur[:kk, k, :],
                                     start=False, stop=(k == 2))
                # hop3's svec + its partition broadcast go BEFORE the attn
                # stream on the PE, so hop3's score chain is not queued
                # behind it
                nc.vector.tensor_scalar_add(svec_sb[:], E3_ps[:],
                                            auxf_sb[0:1, AF_LWVB:AF_LWVB + 1])
                svbc3_ps = svbc_mm()
                for c in vcols:
                    attn_mms(attn_ps, awn[:, c:c + 1], c, opened=True)
                last_attn = attn_ps

            # ======== final hop fused with the output projection ========
            # logits^T = sum_c (emb@out_w)_c^T @ alpha_c
            #          + (lin_w@out_w)^T @ vec_prev + (lin_b@out_w + out_b)
            # The score chain is emitted BEFORE hop2's PSUM readout so the
            # DVE/ACT chain overlaps hop2's attention stream (only the final
            # awn write has a WAR on the stream's alpha reads).
            weights_from_svec(svbc3_ps)
            vfin = vecT_a if N_HOPS % 2 == 1 else vecT_b
            # readout on the ACT engine so the DVE score chain for the final
            # hop is not blocked behind it (GPSIMD cannot access PSUM)
            nc.scalar.activation(vfin[:], last_attn[:], AF.Copy)
            if DEBUG:
                dbg_v2 = sb.tile([P, 3, BP], F32, tag="dbg_v2_t")
                nc.vector.memset(dbg_v2[:], 0.0)
                for k in range(3):
                    nc.vector.tensor_copy(out=dbg_v2[:DK[k], k, :],
                                          in_=vfin[:DK[k], k, :])
                nc.sync.dma_start(dbg_v2_d.ap(), dbg_v2[:])
            lg_ps = ps3.tile([C, BP], F32, tag="psmall", space="PSUM")
            for k in range(3):
                kk = DK[k]
                nc.tensor.matmul(
                    lg_ps[:],
                    lhsT=aux16_sb[:kk, A6_LWOW + k * C:A6_LWOW + (k + 1) * C],
                    rhs=vfin[:kk, k, :], start=(k == 0), stop=False)
            nc.tensor.matmul(lg_ps[:],
                             lhsT=aux16_sb[0:1, A6_OUTB:A6_OUTB + C],
                             rhs=onesr_ap[0:1, 0:BP], start=False, stop=False)
            for c in vcols:
                g, cc = divmod(c, GW)
                b = c % BP
                nc.tensor.matmul(lg_ps[:, b:b + 1],
                                 lhsT=mem_sb[g][:, cc, OW_COL:OW_COL + C],
                                 rhs=awn[:, c:c + 1],
                                 start=False, stop=(c == vcols[-1]))
            nc.vector.tensor_copy(out=lg_sb[:], in_=lg_ps[:])
            nc.sync.dma_start(out_d.ap(), lg_sb[:])

    nc.compile()
    return nc


def _slot_order(context_len):
    """Slot permutation: batches sorted by descending context_len."""
    return np.argsort(-np.asarray(context_len), kind="stable")


def _band_counts(context_len):
    """(m1, m2, m3): #batches with len > 128r for r = 1, 2, 3."""
    cl = np.asarray(context_len)
    return tuple(int((cl > 128 * r).sum()) for r in (1, 2, 3))


def _wrap16(flat):
    """dma_gather index layout: [128, n/16], replicated over 16-row groups."""
    n = flat.shape[0]
    w = flat.reshape(n // 16, 16).T.astype(np.int16)   # [16, n/16]
    return np.ascontiguousarray(np.tile(w, (8, 1)))    # [128, n/16]


def make_core_inputs(context_x, context_len, target_x, target_len, target_loc,
                     emb16, shared):
    """Per-core input dict. context_x etc are the 32-row shards (numpy).

    The embedding table is sharded per core by index compaction: each core
    receives only the (unique) rows its context references, packed as 768B
    rows (300 fp8 values + the precomputed fp16 content score emb@w_mem at
    byte 300), plus int16 local indices in the wrapped dma_gather layout.
    Target rows (160 per core) are materialized host-side in fp16.
    """
    score16 = shared["_score16"]
    order = _slot_order(context_len)
    # flat gather stream in (c, p) order: chunk c = 32r + j holds
    # l in [128r, 128r+128) of batch order[j]
    cj = np.arange(NCH) % BP
    cr = np.arange(NCH) // BP
    bmap = order[cj]                                       # [NCH]
    flat = np.zeros((NCH, P), np.int64)
    for c in range(NCH):
        flat[c] = context_x[bmap[c], cr[c] * P:(cr[c] + 1) * P]
    flat = flat.reshape(-1)
    uniq, inv = np.unique(flat, return_inverse=True)
    assert uniq.shape[0] <= U_PAD
    emb_loc = np.zeros((U_PAD, EPAD16), np.float16)
    emb_loc[:uniq.shape[0], :D] = emb16[uniq]
    emb_loc[:uniq.shape[0], SCORE_COL] = score16[uniq]
    emb_loc[:uniq.shape[0], OW_COL:OW_COL + C] = shared["_ow16"][uniq]
    emb_loc[:uniq.shape[0], WV_COL] = shared["_wv16"][uniq]
    ctx_idx = _wrap16(inv)
    nspl = 4 * GW * P // 16

    # host-materialized target rows: row r = j*128 + p -> t = r//32, b = r%32
    tgtr = np.zeros((P, TCOL, TE), np.float16)
    for j in range(TCOL):
        for t0 in range(P // BP):
            t = j * (P // BP) + t0
            if t >= T:
                break
            rows = emb16[target_x[:, t]]                 # [BP, D]
            tgtr[t0 * BP:(t0 + 1) * BP, j, :D] = rows
    tgtr = tgtr.reshape(P, TCOL * TE)

    # host-side location model per (p, c): c = 32r + j -> b = order[j],
    # l = 128r + p
    pos = (cr[None, :] * P + np.arange(P)[:, None]).astype(np.float64)
    loc_b = target_loc[bmap].astype(np.float64)[None, :]
    len_b = context_len[bmap].astype(np.float64)[None, :]
    vloc = 1.0 - np.abs(pos - loc_b) / len_b
    cmask = (pos < len_b).astype(np.float64)

    auxf = np.zeros((P, AF_N), np.float32)
    auxf[:, AF_TLEN] = target_len[np.arange(P) % BP]
    auxf[:, AF_T0] = np.arange(P) // BP
    auxf[:, AF_T1] = (P // BP) + np.arange(P) // BP
    auxf[:BP, AF_ID32:AF_ID32 + BP] = np.eye(BP)
    auxf[:C, AF_OUTB] = shared["_outb"]
    auxf[0, AF_ATTNB] = shared["_attnb"]
    auxf[:BP, AF_TLENP] = target_len[order]
    auxf[:, AF_SSEL:AF_SSEL + BP] = (
        np.arange(P)[:, None] % BP == order[None, :])
    auxf[0, AF_LWVB] = shared["_lwvb"]

    aux16 = np.zeros((P, A6_N + TCOL * TE), np.float16)
    aux16[:, :A6_N] = shared["aux16_h"]
    aux16[:, A6_VLOC:A6_VLOC + NCH] = vloc
    aux16[:, A6_CMASK:A6_CMASK + NCH] = cmask
    aux16[:, A6_CV:A6_CV + NCH] = cmask * vloc
    aux16[:, A6_N:] = tgtr

    d = dict(aux16_h=aux16, lin_w_h=shared["lin_w_h"])
    d.update(emb_loc=emb_loc,
             ctx_idx16a=np.ascontiguousarray(ctx_idx[:, :nspl]),
             ctx_idx16b=np.ascontiguousarray(ctx_idx[:, nspl:]),
             auxf_h=auxf)
    return d


def make_shared_inputs(emb, attn_w, attn_b, lin_w, lin_b, out_w, out_b):
    lin_w_pad = np.zeros((384, 384), np.float16)
    lin_w_pad[:D, :D] = lin_w.astype(np.float16)
    lin_w_h = np.ascontiguousarray(
        lin_w_pad.reshape(3, P, 384).transpose(1, 0, 2).reshape(P, 3 * 384))

    aux16 = np.zeros((P, A6_N), np.float16)
    # gsel: chunk (partition) c -> slot column c % 32
    aux16[:, A6_GSEL:A6_GSEL + BP] = (
        np.arange(P)[:, None] % BP == np.arange(BP)[None, :])
    w_vec_pad = np.zeros((384,), np.float16)
    w_vec_pad[:D] = attn_w[D:, 0].astype(np.float16)
    aux16[:, A6_WVEC:A6_WVEC + 3] = w_vec_pad.reshape(3, P).T
    aux16[:, A6_ONES] = 1.0
    aux16[0, A6_ONESR:A6_ONESR + P] = 1.0
    out_w_pad = np.zeros((384, C), np.float16)
    out_w_pad[:D] = out_w.astype(np.float16)
    aux16[:, A6_OUTW:A6_OUTW + 3 * C] = (
        out_w_pad.reshape(3, P, C).transpose(1, 0, 2).reshape(P, 3 * C))
    aux16[0, A6_LINB:A6_LINB + D] = lin_b.astype(np.float16)
    aux16[0, A6_OUTB:A6_OUTB + C] = (
        np.asarray(out_b, np.float64)
        + np.asarray(lin_b, np.float64) @ np.asarray(out_w, np.float64)
    ).astype(np.float16)
    lwow = np.asarray(lin_w, np.float64) @ np.asarray(out_w, np.float64)
    lwow_pad = np.zeros((384, C), np.float16)
    lwow_pad[:D] = lwow.astype(np.float16)
    aux16[:, A6_LWOW:A6_LWOW + 3 * C] = (
        lwow_pad.reshape(3, P, C).transpose(1, 0, 2).reshape(P, 3 * C))
    lwv = np.asarray(lin_w, np.float64) @ np.asarray(attn_w[D:, 0], np.float64)
    lwv_pad = np.zeros((384,), np.float16)
    lwv_pad[:D] = lwv.astype(np.float16)
    aux16[:, A6_LWV:A6_LWV + 3] = lwv_pad.reshape(3, P).T

    lin_b_pad = np.zeros((384,), np.float32)
    lin_b_pad[:D] = lin_b
    score16 = (np.asarray(emb, np.float64)
               @ np.asarray(attn_w[:D, 0], np.float64)).astype(np.float16)
    ow16 = (np.asarray(emb, np.float64)
            @ np.asarray(out_w, np.float64)).astype(np.float16)
    wv16 = (np.asarray(emb, np.float64)
            @ np.asarray(attn_w[D:, 0], np.float64)).astype(np.float16)
    lwvb = np.float32(np.asarray(lin_b, np.float64)
                      @ np.asarray(attn_w[D:, 0], np.float64)
                      + np.float64(attn_b[0]))
    outb_eff = (np.asarray(out_b, np.float64)
                + np.asarray(lin_b, np.float64)
                @ np.asarray(out_w, np.float64)).astype(np.float32)
    return dict(
        lin_w_h=lin_w_h,
        aux16_h=aux16,
        _linb3=np.ascontiguousarray(lin_b_pad.reshape(3, P).T),
        _outb=outb_eff,
        _attnb=np.float32(attn_b[0]),
        _score16=score16,
        _ow16=ow16,
        _wv16=wv16,
        _lwvb=lwvb,
    )


_module_cache = {}


def get_module(m=None):
    if m is None:
        # most-recently built module (test.py convenience)
        return next(reversed(_module_cache.values()))
    if m not in _module_cache:
        _module_cache[m] = build_module(m)
    return _module_cache[m]


def kernel(**inputs):
    emb16 = np.ascontiguousarray(inputs["emb"].astype(np.float16))
    shared = make_shared_inputs(
        np.asarray(inputs["emb"]), np.asarray(inputs["attn_w"]),
        np.asarray(inputs["attn_b"]), np.asarray(inputs["lin_w"]),
        np.asarray(inputs["lin_b"]), np.asarray(inputs["out_w"]),
        np.asarray(inputs["out_b"]))
    context_len = np.asarray(inputs["context_len"])
    in_maps = []
    orders = []
    mm = (0, 0, 0)
    for k in range(N_CORES):
        s = slice(k * BP, (k + 1) * BP)
        in_maps.append(make_core_inputs(
            np.asarray(inputs["context_x"])[s],
            context_len[s],
            np.asarray(inputs["target_x"])[s],
            np.asarray(inputs["target_len"])[s],
            np.asarray(inputs["target_loc"])[s],
            emb16, shared))
        orders.append(_slot_order(context_len[s]))
        mm = tuple(max(a, b) for a, b in
                   zip(mm, _band_counts(context_len[s])))
    nc = get_module(mm)
    res = bass_utils.run_bass_kernel_spmd(nc, in_maps,
                                          core_ids=list(range(N_CORES)))
    out = np.empty((B, C), np.float32)
    for k in range(N_CORES):
        blk = res.results[k]["logits_t"].T.astype(np.float32)  # [slot, C]
        out[k * BP + orders[k]] = blk
    return out
